# revision 5
# baseline (speedup 1.0000x reference)
"""ColAttention TRN2 kernel v2: out = gamma * colattn(x) + x.

Sharding: width. Core k gets x[:, :, :, 16k:16(k+1)], host-transposed to
(B, C, WT, H) bf16 so every on-device slice is contiguous along h.
Per core: 8 batches x 16 width columns = 128 independent attention
problems over h=128.

All-bf16 datapath (PSUM accumulation in f32):
  QK proj: bf16 matmuls over (w h) tiles, PSUM-accumulated over 4 c-chunks
  per (b,w): V^T (h,c) via 4 bf16 MMs (lhsT = x chunk, contiguous -> FWL)
             S(i,j) = Q^T K (bf16, N=128)
             exp + row-sums (ACT accum_out); attn = exp * (1/sums) -> bf16
             attn^T via PE transpose; AV: out(c,i) = V^T.T @ attn_T
             residual: out = (AV + gamma*bv) + x, split DVE / GpSimd,
             contiguous APs, into a separate (c, w, h) f32 output slab
Output HBM layout (B, C, WT, H); host transposes back to (B, C, H, WT).
"""

import numpy as np
import ml_dtypes

import concourse.bass as bass
from concourse import bacc, mybir
from concourse.tile import TileContext
from concourse.bass_utils import run_bass_kernel_spmd

f32 = mybir.dt.float32
bf16 = mybir.dt.bfloat16
AF = mybir.ActivationFunctionType
ALU = mybir.AluOpType

N_CORES = 8
B, C, H, W = 8, 512, 128, 128
WT = W // N_CORES          # 16 w-columns per core
DQ = 64
NCH = C // 128             # 4 c-chunks
WH = WT * H                # 2048 free elems per chunk slab

TRACE = False              # set True from test.py for profiling
LAST_RESULTS = None


def _build(bqk_is_zero: bool, bv_is_zero: bool):
    nc = bacc.Bacc("TRN2", num_devices=N_CORES, debug=False)

    x_d = nc.dram_tensor("x", (B, C, WT, H), bf16, kind="ExternalInput")
    wqk_d = nc.dram_tensor("wqkT", (C, 128), bf16, kind="ExternalInput")
    bqk_d = nc.dram_tensor("bqk", (128, 1), f32, kind="ExternalInput")
    wv_d = nc.dram_tensor("wvT", (C, C), bf16, kind="ExternalInput")
    gbv_d = nc.dram_tensor("gbv", (128, NCH), f32, kind="ExternalInput")
    out_d = nc.dram_tensor("out", (B, C, WT, H), f32, kind="ExternalOutput")
    id_d = nc.inline_tensor(np.eye(128, dtype=ml_dtypes.bfloat16), name="id128")

    xa = x_d.ap()
    oa = out_d.ap()

    with TileContext(nc) as tc:
        with (
            tc.tile_pool(name="const", bufs=1) as cpool,
            tc.tile_pool(name="xs", bufs=2) as xspool,
            tc.tile_pool(name="os", bufs=2) as ospool,
            tc.tile_pool(name="qk", bufs=2) as qkpool,
            tc.tile_pool(name="small", bufs=3) as spool,
            tc.tile_pool(name="pqk", bufs=1, space="PSUM") as pqk,
            tc.tile_pool(name="pvt", bufs=2, space="PSUM") as pvt,
            tc.tile_pool(name="psmall", bufs=3, space="PSUM") as psmall,
            tc.tile_pool(name="pav", bufs=2, space="PSUM") as pav,
        ):
            # ---- constants ----
            wqk_sb = cpool.tile([128, 128 * NCH], bf16, name="wqk_sb")
            for ci in range(NCH):
                nc.sync.dma_start(wqk_sb[:, ci * 128:(ci + 1) * 128],
                                  wqk_d.ap()[ci * 128:(ci + 1) * 128, :])
            wv_sb = cpool.tile([128, 512 * NCH], bf16, name="wv_sb")
            for ci in range(NCH):
                nc.sync.dma_start(wv_sb[:, ci * 512:(ci + 1) * 512],
                                  wv_d.ap()[ci * 128:(ci + 1) * 128, :])
            bqk_sb = cpool.tile([128, 1], f32, name="bqk_sb")
            nc.sync.dma_start(bqk_sb[:], bqk_d.ap())
            gbv_sb = cpool.tile([128, NCH], f32, name="gbv_sb")
            nc.sync.dma_start(gbv_sb[:], gbv_d.ap())
            id_sb = cpool.tile([128, 128], bf16, name="id_sb")
            nc.sync.dma_start(id_sb[:], id_d.ap())

            for b in range(B):
                # ---- batch prologue: hoisted into previous batch's w-loop ----
                with tc.high_priority(offset=0 if b == 0 else 200):
                    # load slab (4 chunks, contiguous 512 KiB each)
                    xs = xspool.tile([128, NCH * WH], bf16, tag="xs", name=f"xs{b}")
                    xs4 = xs[:].rearrange("p (c w h) -> p c w h", c=NCH, w=WT)
                    for ci in range(NCH):
                        nc.sync.dma_start(xs4[:, ci], xa[b, ci * 128:(ci + 1) * 128])

                    # output slab (separate, f32)
                    os_ = ospool.tile([128, NCH * WH], f32, tag="os", name=f"os{b}")
                    o4 = os_[:].rearrange("p (c w h) -> p c w h", c=NCH, w=WT)

                    # QK projection over (w h), n-tiles of 512
                    qk_sb = qkpool.tile([128, WH], bf16, tag="qk", name=f"qk{b}")
                    ks = qkpool.tile([64, WH], bf16, tag="ks", name=f"ks{b}")
                    for nt in range(WH // 512):
                        qkp = pqk.tile([128, 512], f32, tag="qkp")
                        for ci in range(NCH):
                            nc.tensor.matmul(
                                qkp[:],
                                wqk_sb[:, ci * 128:(ci + 1) * 128],
                                xs[:, ci * WH + nt * 512: ci * WH + (nt + 1) * 512],
                                start=(ci == 0), stop=(ci == NCH - 1))
                        dst = qk_sb[:, nt * 512:(nt + 1) * 512]
                        if bqk_is_zero:
                            if nt % 2 == 0:
                                nc.scalar.activation(dst, qkp[:], AF.Identity)
                            else:
                                nc.vector.tensor_copy(dst, qkp[:])
                        else:
                            nc.scalar.activation(dst, qkp[:], AF.Identity,
                                                 bias=bqk_sb[:])
                        # K rows 64:128 -> partitions 0:63 (scores needs same base)
                        nc.sync.dma_start(ks[:, nt * 512:(nt + 1) * 512],
                                          qk_sb[64:128, nt * 512:(nt + 1) * 512])
                qk3 = qk_sb[:].rearrange("p (w h) -> p w h", w=WT)
                ks3 = ks[:].rearrange("p (w h) -> p w h", w=WT)

                for w in range(WT):
                    # ---- V^T_w (h, c): lhsT = x chunk (contiguous -> FWL) ----
                    vt = pvt.tile([128, 512], f32, tag="vt")
                    for ci in range(NCH):
                        nc.tensor.matmul(vt[:], xs4[:, ci, w, :],
                                         wv_sb[:, ci * 512:(ci + 1) * 512],
                                         start=(ci == 0), stop=(ci == NCH - 1))
                    v_sb = spool.tile([128, 512], bf16, tag="v_sb")
                    if w % 2 == 0:
                        nc.scalar.activation(v_sb[:], vt[:], AF.Identity)
                    else:
                        nc.vector.tensor_copy(v_sb[:], vt[:])

                    # ---- scores S(i,j), k=64, bf16 ----
                    sc = psmall.tile([128, 128], f32, tag="small")
                    nc.tensor.matmul(sc[:], qk3[0:64, w, :], ks3[:, w, :],
                                     start=True, stop=True)

                    # ---- softmax (unnormalized exp + row sums) ----
                    ex = spool.tile([128, 128], bf16, tag="ex")
                    sums = spool.tile([128, 1], f32, tag="sums")
                    nc.scalar.activation(ex[:], sc[:], AF.Exp, accum_out=sums[:])
                    rr = spool.tile([128, 1], f32, tag="rr")
                    nc.vector.reciprocal(rr[:], sums[:])
                    at = spool.tile([128, 128], bf16, tag="at")
                    nc.gpsimd.tensor_scalar_mul(at[:], ex[:], rr[:])

                    # ---- attn^T via PE transpose ----
                    atp = psmall.tile([128, 128], bf16, tag="small")
                    nc.tensor.transpose(atp[:], at[:], id_sb[:])
                    ats = spool.tile([128, 128], bf16, tag="ats")
                    if w % 2 == 0:
                        nc.scalar.activation(ats[:], atp[:], AF.Identity)
                    else:
                        nc.vector.tensor_copy(ats[:], atp[:])

                    # ---- AV: out(c, i) per c-chunk into one bank ----
                    av = pav.tile([128, 512], f32, tag="av")
                    for ci in range(NCH):
                        nc.tensor.matmul(av[:, ci * 128:(ci + 1) * 128],
                                         v_sb[:, ci * 128:(ci + 1) * 128],
                                         ats[:], start=True, stop=True)

                    # ---- final: out = (AV + gamma*bv) + x, contiguous ----
                    av3 = av[:].rearrange("p (c h) -> p c h", c=NCH)
                    if bv_is_zero:
                        nc.vector.scalar_tensor_tensor(
                            o4[:, :, w, :], av3, 0.0,
                            xs4[:, :, w, :], ALU.add, ALU.add)
                    else:
                        for ci in range(NCH):
                            nc.vector.scalar_tensor_tensor(
                                o4[:, ci, w, :], av3[:, ci],
                                gbv_sb[:, ci:ci + 1],
                                xs4[:, ci, w, :], ALU.add, ALU.add)

                # ---- store slab ----
                for ci in range(NCH):
                    nc.sync.dma_start(oa[b, ci * 128:(ci + 1) * 128], o4[:, ci])

    nc.compile()
    return nc


def kernel(x, Wq, bq, Wk, bk, Wv, bv, gamma):
    global LAST_RESULTS
    x = np.asarray(x, dtype=np.float32)
    Wq = np.asarray(Wq, dtype=np.float32)
    bq = np.asarray(bq, dtype=np.float32)
    Wk = np.asarray(Wk, dtype=np.float32)
    bk = np.asarray(bk, dtype=np.float32)
    Wv = np.asarray(Wv, dtype=np.float32)
    bv = np.asarray(bv, dtype=np.float32)
    g = float(np.asarray(gamma, dtype=np.float32).reshape(-1)[0])

    bqk_is_zero = not (np.any(bq) or np.any(bk))
    bv_is_zero = not np.any(bv)
    nc = _build(bqk_is_zero, bv_is_zero)

    wqkT = np.ascontiguousarray(
        np.concatenate([Wq, Wk], axis=0).T).astype(ml_dtypes.bfloat16)
    bqk = np.concatenate([bq, bk], axis=0).reshape(128, 1)
    wvT = np.ascontiguousarray((g * Wv).T).astype(ml_dtypes.bfloat16)
    gbv = np.ascontiguousarray((g * bv).reshape(NCH, 128).T)

    # (B, C, W, H) bf16, then per-core contiguous slices
    xt = np.ascontiguousarray(x.transpose(0, 1, 3, 2)).astype(ml_dtypes.bfloat16)

    in_maps = []
    for k in range(N_CORES):
        in_maps.append({
            "x": np.ascontiguousarray(xt[:, :, k * WT:(k + 1) * WT, :]),
            "wqkT": wqkT,
            "bqk": bqk,
            "wvT": wvT,
            "gbv": gbv,
        })

    res = run_bass_kernel_spmd(nc, in_maps, core_ids=list(range(N_CORES)),
                               trace=TRACE)
    LAST_RESULTS = res

    out = np.empty((B, C, H, W), dtype=np.float32)
    for k in range(N_CORES):
        out[:, :, :, k * WT:(k + 1) * WT] = res.results[k]["out"].transpose(0, 1, 3, 2)
    return out


# revision 6
# speedup vs baseline: 1.5753x; 1.5753x over previous
"""ColAttention TRN2 kernel v2: out = gamma * colattn(x) + x.

Sharding: width. Core k gets x[:, :, :, 16k:16(k+1)], host-transposed to
(B, C, WT, H) bf16 so every on-device slice is contiguous along h.
Per core: 8 batches x 16 width columns = 128 independent attention
problems over h=128.

All-bf16 datapath (PSUM accumulation in f32):
  QK proj: bf16 matmuls over (w h) tiles, PSUM-accumulated over 4 c-chunks
  per (b,w): V^T (h,c) via 4 bf16 MMs (lhsT = x chunk, contiguous -> FWL)
             S(i,j) = Q^T K (bf16, N=128)
             exp + row-sums (ACT accum_out); attn = exp * (1/sums) -> bf16
             attn^T via PE transpose; AV: out(c,i) = V^T.T @ attn_T
             residual: out = (AV + gamma*bv) + x, split DVE / GpSimd,
             contiguous APs, into a separate (c, w, h) f32 output slab
Output HBM layout (B, C, WT, H); host transposes back to (B, C, H, WT).
"""

import numpy as np
import ml_dtypes

import concourse.bass as bass
from concourse import bacc, mybir
from concourse.tile import TileContext
from concourse.bass_utils import run_bass_kernel_spmd

f32 = mybir.dt.float32
bf16 = mybir.dt.bfloat16
AF = mybir.ActivationFunctionType
ALU = mybir.AluOpType

N_CORES = 8
B, C, H, W = 8, 512, 128, 128
WT = W // N_CORES          # 16 w-columns per core
DQ = 64
NCH = C // 128             # 4 c-chunks
WH = WT * H                # 2048 free elems per chunk slab

TRACE = False              # set True from test.py for profiling
LAST_RESULTS = None


def _build(bqk_is_zero: bool, bv_is_zero: bool):
    nc = bacc.Bacc("TRN2", num_devices=N_CORES, debug=False)

    x_d = nc.dram_tensor("x", (B, C, WT, H), bf16, kind="ExternalInput")
    wqk_d = nc.dram_tensor("wqkT", (C, 128), bf16, kind="ExternalInput")
    bqk_d = nc.dram_tensor("bqk", (128, 1), f32, kind="ExternalInput")
    wv_d = nc.dram_tensor("wvT", (C, C), bf16, kind="ExternalInput")
    gbv_d = nc.dram_tensor("gbv", (128, NCH), f32, kind="ExternalInput")
    out_d = nc.dram_tensor("out", (B, C, WT, H), f32, kind="ExternalOutput")
    id_d = nc.inline_tensor(np.eye(128, dtype=ml_dtypes.bfloat16), name="id128")

    xa = x_d.ap()
    oa = out_d.ap()

    with TileContext(nc) as tc:
        with (
            tc.tile_pool(name="const", bufs=1) as cpool,
            tc.tile_pool(name="xs", bufs=2) as xspool,
            tc.tile_pool(name="os", bufs=2) as ospool,
            tc.tile_pool(name="qk", bufs=2) as qkpool,
            tc.tile_pool(name="small", bufs=3) as spool,
            tc.tile_pool(name="pqk", bufs=1, space="PSUM") as pqk,
            tc.tile_pool(name="pvt", bufs=2, space="PSUM") as pvt,
            tc.tile_pool(name="psmall", bufs=3, space="PSUM") as psmall,
            tc.tile_pool(name="pav", bufs=2, space="PSUM") as pav,
        ):
            # ---- constants ----
            wqk_sb = cpool.tile([128, 128 * NCH], bf16, name="wqk_sb")
            for ci in range(NCH):
                nc.sync.dma_start(wqk_sb[:, ci * 128:(ci + 1) * 128],
                                  wqk_d.ap()[ci * 128:(ci + 1) * 128, :])
            wv_sb = cpool.tile([128, 512 * NCH], bf16, name="wv_sb")
            for ci in range(NCH):
                nc.sync.dma_start(wv_sb[:, ci * 512:(ci + 1) * 512],
                                  wv_d.ap()[ci * 128:(ci + 1) * 128, :])
            bqk_sb = cpool.tile([128, 1], f32, name="bqk_sb")
            nc.sync.dma_start(bqk_sb[:], bqk_d.ap())
            gbv_sb = cpool.tile([128, NCH], f32, name="gbv_sb")
            nc.sync.dma_start(gbv_sb[:], gbv_d.ap())
            id_sb = cpool.tile([128, 128], bf16, name="id_sb")
            nc.sync.dma_start(id_sb[:], id_d.ap())

            for b in range(B):
                # ---- batch prologue: hoisted into previous batch's w-loop ----
                with tc.high_priority(offset=0 if b == 0 else 200):
                    # load slab (4 chunks, contiguous 512 KiB each)
                    xs = xspool.tile([128, NCH * WH], bf16, tag="xs", name=f"xs{b}")
                    xs4 = xs[:].rearrange("p (c w h) -> p c w h", c=NCH, w=WT)
                    for ci in range(NCH):
                        nc.sync.dma_start(xs4[:, ci], xa[b, ci * 128:(ci + 1) * 128])

                    # output slab (separate, f32)
                    os_ = ospool.tile([128, NCH * WH], f32, tag="os", name=f"os{b}")
                    o4 = os_[:].rearrange("p (c w h) -> p c w h", c=NCH, w=WT)

                    # QK projection over (w h), n-tiles of 512
                    qk_sb = qkpool.tile([128, WH], bf16, tag="qk", name=f"qk{b}")
                    ks = qkpool.tile([64, WH], bf16, tag="ks", name=f"ks{b}")
                    for nt in range(WH // 512):
                        qkp = pqk.tile([128, 512], f32, tag="qkp")
                        for ci in range(NCH):
                            nc.tensor.matmul(
                                qkp[:],
                                wqk_sb[:, ci * 128:(ci + 1) * 128],
                                xs[:, ci * WH + nt * 512: ci * WH + (nt + 1) * 512],
                                start=(ci == 0), stop=(ci == NCH - 1))
                        dst = qk_sb[:, nt * 512:(nt + 1) * 512]
                        if bqk_is_zero:
                            if nt % 2 == 0:
                                nc.scalar.activation(dst, qkp[:], AF.Identity)
                            else:
                                nc.vector.tensor_copy(dst, qkp[:])
                        else:
                            nc.scalar.activation(dst, qkp[:], AF.Identity,
                                                 bias=bqk_sb[:])
                        # K rows 64:128 -> partitions 0:63 (scores needs same base)
                        nc.sync.dma_start(ks[:, nt * 512:(nt + 1) * 512],
                                          qk_sb[64:128, nt * 512:(nt + 1) * 512])
                qk3 = qk_sb[:].rearrange("p (w h) -> p w h", w=WT)
                ks3 = ks[:].rearrange("p (w h) -> p w h", w=WT)

                for w in range(WT):
                    # ---- V^T_w (h, c): lhsT = x chunk (contiguous -> FWL) ----
                    vt = pvt.tile([128, 512], f32, tag="vt")
                    for ci in range(NCH):
                        nc.tensor.matmul(vt[:], xs4[:, ci, w, :],
                                         wv_sb[:, ci * 512:(ci + 1) * 512],
                                         start=(ci == 0), stop=(ci == NCH - 1))
                    v_sb = spool.tile([128, 512], bf16, tag="v_sb")
                    if w % 2 == 0:
                        nc.scalar.activation(v_sb[:], vt[:], AF.Identity)
                    else:
                        nc.vector.tensor_copy(v_sb[:], vt[:])

                    # ---- scores S(i,j), k=64, bf16 ----
                    sc = psmall.tile([128, 128], f32, tag="small")
                    nc.tensor.matmul(sc[:], qk3[0:64, w, :], ks3[:, w, :],
                                     start=True, stop=True)

                    # ---- softmax (unnormalized exp + row sums) ----
                    ex = spool.tile([128, 128], bf16, tag="ex")
                    sums = spool.tile([128, 1], f32, tag="sums")
                    nc.scalar.activation(ex[:], sc[:], AF.Exp, accum_out=sums[:])
                    rr = spool.tile([128, 1], f32, tag="rr")
                    nc.vector.reciprocal(rr[:], sums[:])
                    at = spool.tile([128, 128], bf16, tag="at")
                    nc.vector.tensor_scalar_mul(at[:], ex[:], rr[:])

                    # ---- attn^T via PE transpose ----
                    atp = psmall.tile([128, 128], bf16, tag="small")
                    nc.tensor.transpose(atp[:], at[:], id_sb[:])
                    ats = spool.tile([128, 128], bf16, tag="ats")
                    if w % 2 == 0:
                        nc.scalar.activation(ats[:], atp[:], AF.Identity)
                    else:
                        nc.vector.tensor_copy(ats[:], atp[:])

                    # ---- AV: out(c, i) per c-chunk into one bank ----
                    av = pav.tile([128, 512], f32, tag="av")
                    for ci in range(NCH):
                        nc.tensor.matmul(av[:, ci * 128:(ci + 1) * 128],
                                         v_sb[:, ci * 128:(ci + 1) * 128],
                                         ats[:], start=True, stop=True)

                    # ---- final: out = (AV + gamma*bv) + x, contiguous ----
                    av3 = av[:].rearrange("p (c h) -> p c h", c=NCH)
                    if bv_is_zero:
                        nc.vector.scalar_tensor_tensor(
                            o4[:, :, w, :], av3, 0.0,
                            xs4[:, :, w, :], ALU.add, ALU.add)
                    else:
                        for ci in range(NCH):
                            nc.vector.scalar_tensor_tensor(
                                o4[:, ci, w, :], av3[:, ci],
                                gbv_sb[:, ci:ci + 1],
                                xs4[:, ci, w, :], ALU.add, ALU.add)

                # ---- store slab ----
                for ci in range(NCH):
                    nc.sync.dma_start(oa[b, ci * 128:(ci + 1) * 128], o4[:, ci])

    nc.compile()
    return nc


def kernel(x, Wq, bq, Wk, bk, Wv, bv, gamma):
    global LAST_RESULTS
    x = np.asarray(x, dtype=np.float32)
    Wq = np.asarray(Wq, dtype=np.float32)
    bq = np.asarray(bq, dtype=np.float32)
    Wk = np.asarray(Wk, dtype=np.float32)
    bk = np.asarray(bk, dtype=np.float32)
    Wv = np.asarray(Wv, dtype=np.float32)
    bv = np.asarray(bv, dtype=np.float32)
    g = float(np.asarray(gamma, dtype=np.float32).reshape(-1)[0])

    bqk_is_zero = not (np.any(bq) or np.any(bk))
    bv_is_zero = not np.any(bv)
    nc = _build(bqk_is_zero, bv_is_zero)

    wqkT = np.ascontiguousarray(
        np.concatenate([Wq, Wk], axis=0).T).astype(ml_dtypes.bfloat16)
    bqk = np.concatenate([bq, bk], axis=0).reshape(128, 1)
    wvT = np.ascontiguousarray((g * Wv).T).astype(ml_dtypes.bfloat16)
    gbv = np.ascontiguousarray((g * bv).reshape(NCH, 128).T)

    # (B, C, W, H) bf16, then per-core contiguous slices
    xt = np.ascontiguousarray(x.transpose(0, 1, 3, 2)).astype(ml_dtypes.bfloat16)

    in_maps = []
    for k in range(N_CORES):
        in_maps.append({
            "x": np.ascontiguousarray(xt[:, :, k * WT:(k + 1) * WT, :]),
            "wqkT": wqkT,
            "bqk": bqk,
            "wvT": wvT,
            "gbv": gbv,
        })

    res = run_bass_kernel_spmd(nc, in_maps, core_ids=list(range(N_CORES)),
                               trace=TRACE)
    LAST_RESULTS = res

    out = np.empty((B, C, H, W), dtype=np.float32)
    for k in range(N_CORES):
        out[:, :, :, k * WT:(k + 1) * WT] = res.results[k]["out"].transpose(0, 1, 3, 2)
    return out


# revision 10
# speedup vs baseline: 2.3507x; 1.4922x over previous
"""ColAttention TRN2 kernel v2: out = gamma * colattn(x) + x.

Sharding: width. Core k gets x[:, :, :, 16k:16(k+1)], host-transposed to
(B, C, WT, H) bf16 so every on-device slice is contiguous along h.
Per core: 8 batches x 16 width columns = 128 independent attention
problems over h=128.

All-bf16 datapath (PSUM accumulation in f32):
  QK proj: bf16 matmuls over (w h) tiles, PSUM-accumulated over 4 c-chunks
  per (b,w): V^T (h,c) via 4 bf16 MMs (lhsT = x chunk, contiguous -> FWL)
             S(i,j) = Q^T K (bf16, N=128)
             exp + row-sums (ACT accum_out); attn = exp * (1/sums) -> bf16
             attn^T via PE transpose; AV: out(c,i) = V^T.T @ attn_T
             residual: out = (AV + gamma*bv) + x, split DVE / GpSimd,
             contiguous APs, into a separate (c, w, h) f32 output slab
Output HBM layout (B, C, WT, H); host transposes back to (B, C, H, WT).
"""

import numpy as np
import ml_dtypes

import concourse.bass as bass
from concourse import bacc, mybir
from concourse.tile import TileContext
from concourse.bass_utils import run_bass_kernel_spmd

f32 = mybir.dt.float32
bf16 = mybir.dt.bfloat16
AF = mybir.ActivationFunctionType
ALU = mybir.AluOpType

N_CORES = 8
B, C, H, W = 8, 512, 128, 128
WT = W // N_CORES          # 16 w-columns per core
DQ = 64
NCH = C // 128             # 4 c-chunks
WH = WT * H                # 2048 free elems per chunk slab

TRACE = False              # set True from test.py for profiling
LAST_RESULTS = None


def _build(bqk_is_zero: bool, bv_is_zero: bool):
    nc = bacc.Bacc("TRN2", num_devices=N_CORES, debug=False)

    x_d = nc.dram_tensor("x", (B, C, WT, H), bf16, kind="ExternalInput")
    wqk_d = nc.dram_tensor("wqkT", (C, 128), bf16, kind="ExternalInput")
    bqk_d = nc.dram_tensor("bqk", (128, 1), f32, kind="ExternalInput")
    wv_d = nc.dram_tensor("wvT", (C, C), bf16, kind="ExternalInput")
    gbv_d = nc.dram_tensor("gbv", (128, NCH), f32, kind="ExternalInput")
    out_d = nc.dram_tensor("out", (B, C, WT, H), f32, kind="ExternalOutput")
    id_d = nc.inline_tensor(np.eye(128, dtype=ml_dtypes.bfloat16), name="id128")

    xa = x_d.ap()
    oa = out_d.ap()

    with TileContext(nc) as tc:
        with (
            tc.tile_pool(name="const", bufs=1) as cpool,
            tc.tile_pool(name="xs", bufs=2) as xspool,
            tc.tile_pool(name="os", bufs=2) as ospool,
            tc.tile_pool(name="qk", bufs=2) as qkpool,
            tc.tile_pool(name="small", bufs=4) as spool,
            tc.tile_pool(name="pqk", bufs=1, space="PSUM") as pqk,
            tc.tile_pool(name="pvt", bufs=2, space="PSUM") as pvt,
            tc.tile_pool(name="psc", bufs=2, space="PSUM") as psc,
            tc.tile_pool(name="ptp", bufs=1, space="PSUM") as ptp,
            tc.tile_pool(name="pav", bufs=2, space="PSUM") as pav,
        ):
            # ---- constants ----
            wqk_sb = cpool.tile([128, 128 * NCH], bf16, name="wqk_sb")
            for ci in range(NCH):
                nc.sync.dma_start(wqk_sb[:, ci * 128:(ci + 1) * 128],
                                  wqk_d.ap()[ci * 128:(ci + 1) * 128, :])
            wv_sb = cpool.tile([128, 512 * NCH], bf16, name="wv_sb")
            for ci in range(NCH):
                nc.sync.dma_start(wv_sb[:, ci * 512:(ci + 1) * 512],
                                  wv_d.ap()[ci * 128:(ci + 1) * 128, :])
            bqk_sb = cpool.tile([128, 1], f32, name="bqk_sb")
            nc.sync.dma_start(bqk_sb[:], bqk_d.ap())
            gbv_sb = cpool.tile([128, NCH], f32, name="gbv_sb")
            nc.sync.dma_start(gbv_sb[:], gbv_d.ap())
            id_sb = cpool.tile([128, 128], bf16, name="id_sb")
            nc.sync.dma_start(id_sb[:], id_d.ap())

            for b in range(B):
                # ---- batch prologue: hoisted into previous batch's w-loop ----
                with tc.high_priority(offset=0 if b == 0 else 200):
                    # load slab (4 chunks, contiguous 512 KiB each)
                    xs = xspool.tile([128, NCH * WH], bf16, tag="xs", name=f"xs{b}")
                    xs4 = xs[:].rearrange("p (c w h) -> p c w h", c=NCH, w=WT)
                    for ci in range(NCH):
                        nc.sync.dma_start(xs4[:, ci], xa[b, ci * 128:(ci + 1) * 128])

                    # output slab (separate, f32)
                    os_ = ospool.tile([128, NCH * WH], f32, tag="os", name=f"os{b}")
                    o4 = os_[:].rearrange("p (c w h) -> p c w h", c=NCH, w=WT)

                    # QK projection over (w h), n-tiles of 512
                    qk_sb = qkpool.tile([128, WH], bf16, tag="qk", name=f"qk{b}")
                    ks = qkpool.tile([64, WH], bf16, tag="ks", name=f"ks{b}")
                    for nt in range(WH // 512):
                        qkp = pqk.tile([128, 512], f32, tag="qkp")
                        for ci in range(NCH):
                            nc.tensor.matmul(
                                qkp[:],
                                wqk_sb[:, ci * 128:(ci + 1) * 128],
                                xs[:, ci * WH + nt * 512: ci * WH + (nt + 1) * 512],
                                start=(ci == 0), stop=(ci == NCH - 1))
                        dst = qk_sb[:, nt * 512:(nt + 1) * 512]
                        if bqk_is_zero:
                            nc.scalar.activation(dst, qkp[:], AF.Identity)
                        else:
                            nc.scalar.activation(dst, qkp[:], AF.Identity,
                                                 bias=bqk_sb[:])
                        # K rows 64:128 -> partitions 0:63 (scores needs same base)
                        nc.sync.dma_start(ks[:, nt * 512:(nt + 1) * 512],
                                          qk_sb[64:128, nt * 512:(nt + 1) * 512])
                qk3 = qk_sb[:].rearrange("p (w h) -> p w h", w=WT)
                ks3 = ks[:].rearrange("p (w h) -> p w h", w=WT)

                for wp in range(WT // 2):
                    # ---- scores for the column pair, one PSUM bank ----
                    sc2 = psc.tile([128, 256], f32, tag="sc2")
                    for k in range(2):
                        w = 2 * wp + k
                        nc.tensor.matmul(sc2[:, k * 128:(k + 1) * 128],
                                         qk3[0:64, w, :], ks3[:, w, :],
                                         start=True, stop=True)

                    # ---- softmax: exp per col (accum sums), recip per pair ----
                    sums2 = spool.tile([128, 2], f32, tag="sums2")
                    ex2 = spool.tile([128, 256], bf16, tag="ex2")
                    for k in range(2):
                        nc.scalar.activation(ex2[:, k * 128:(k + 1) * 128],
                                             sc2[:, k * 128:(k + 1) * 128],
                                             AF.Exp, accum_out=sums2[:, k:k + 1])
                    rr2 = spool.tile([128, 2], f32, tag="rr2")
                    nc.vector.reciprocal(rr2[:], sums2[:])
                    at2 = spool.tile([128, 256], bf16, tag="at2")
                    for k in range(2):
                        nc.vector.tensor_scalar_mul(
                            at2[:, k * 128:(k + 1) * 128],
                            ex2[:, k * 128:(k + 1) * 128], rr2[:, k:k + 1])

                    # ---- attn^T via PE transpose (pair shares one bank) ----
                    atp2 = ptp.tile([128, 256], bf16, tag="atp2")
                    for k in range(2):
                        nc.tensor.transpose(atp2[:, k * 128:(k + 1) * 128],
                                            at2[:, k * 128:(k + 1) * 128],
                                            id_sb[:])
                    ats2 = spool.tile([128, 256], bf16, tag="ats2")
                    nc.scalar.activation(ats2[:], atp2[:], AF.Identity)

                    for k in range(2):
                        w = 2 * wp + k
                        # ---- V^T_w (h, c): lhsT = x chunk (contiguous) ----
                        vt = pvt.tile([128, 512], f32, tag="vt")
                        for ci in range(NCH):
                            nc.tensor.matmul(vt[:], xs4[:, ci, w, :],
                                             wv_sb[:, ci * 512:(ci + 1) * 512],
                                             start=(ci == 0), stop=(ci == NCH - 1))
                        v_sb = spool.tile([128, 512], bf16, tag="v_sb")
                        if w % 2 == 0:
                            nc.scalar.activation(v_sb[:], vt[:], AF.Identity)
                        else:
                            nc.vector.tensor_copy(v_sb[:], vt[:])

                        # ---- AV: out(c, i) per c-chunk into one bank ----
                        av = pav.tile([128, 512], f32, tag="av")
                        for ci in range(NCH):
                            nc.tensor.matmul(av[:, ci * 128:(ci + 1) * 128],
                                             v_sb[:, ci * 128:(ci + 1) * 128],
                                             ats2[:, k * 128:(k + 1) * 128],
                                             start=True, stop=True)

                        # ---- final: out = (AV + gamma*bv) + x, contiguous ----
                        av3 = av[:].rearrange("p (c h) -> p c h", c=NCH)
                        if bv_is_zero:
                            nc.vector.scalar_tensor_tensor(
                                o4[:, :, w, :], av3, 0.0,
                                xs4[:, :, w, :], ALU.add, ALU.add)
                        else:
                            for ci in range(NCH):
                                nc.vector.scalar_tensor_tensor(
                                    o4[:, ci, w, :], av3[:, ci],
                                    gbv_sb[:, ci:ci + 1],
                                    xs4[:, ci, w, :], ALU.add, ALU.add)

                # ---- store slab ----
                for ci in range(NCH):
                    nc.sync.dma_start(oa[b, ci * 128:(ci + 1) * 128], o4[:, ci])

    nc.compile()
    return nc


def kernel(x, Wq, bq, Wk, bk, Wv, bv, gamma):
    global LAST_RESULTS
    x = np.asarray(x, dtype=np.float32)
    Wq = np.asarray(Wq, dtype=np.float32)
    bq = np.asarray(bq, dtype=np.float32)
    Wk = np.asarray(Wk, dtype=np.float32)
    bk = np.asarray(bk, dtype=np.float32)
    Wv = np.asarray(Wv, dtype=np.float32)
    bv = np.asarray(bv, dtype=np.float32)
    g = float(np.asarray(gamma, dtype=np.float32).reshape(-1)[0])

    bqk_is_zero = not (np.any(bq) or np.any(bk))
    bv_is_zero = not np.any(bv)
    nc = _build(bqk_is_zero, bv_is_zero)

    wqkT = np.ascontiguousarray(
        np.concatenate([Wq, Wk], axis=0).T).astype(ml_dtypes.bfloat16)
    bqk = np.concatenate([bq, bk], axis=0).reshape(128, 1)
    wvT = np.ascontiguousarray((g * Wv).T).astype(ml_dtypes.bfloat16)
    gbv = np.ascontiguousarray((g * bv).reshape(NCH, 128).T)

    # (B, C, W, H) bf16, then per-core contiguous slices
    xt = np.ascontiguousarray(x.transpose(0, 1, 3, 2)).astype(ml_dtypes.bfloat16)

    in_maps = []
    for k in range(N_CORES):
        in_maps.append({
            "x": np.ascontiguousarray(xt[:, :, k * WT:(k + 1) * WT, :]),
            "wqkT": wqkT,
            "bqk": bqk,
            "wvT": wvT,
            "gbv": gbv,
        })

    res = run_bass_kernel_spmd(nc, in_maps, core_ids=list(range(N_CORES)),
                               trace=TRACE)
    LAST_RESULTS = res

    out = np.empty((B, C, H, W), dtype=np.float32)
    for k in range(N_CORES):
        out[:, :, :, k * WT:(k + 1) * WT] = res.results[k]["out"].transpose(0, 1, 3, 2)
    return out
